# revision 2
# baseline (speedup 1.0000x reference)
"""GCN encoder (3-layer GCNConv + LayerNorm + ReLU + residual) on 8 TRN2
NeuronCores via Bass/Tile.

Sharding: nodes are partitioned across cores (graph parallel). Each core owns
NPC nodes; per-layer the full (dinv-scaled) xw table is AllGathered to every
core's DRAM, then each core pulls its in-edge source rows with dma_gather,
scales by edge weight, and segment-reduces into its owned destinations.
"""

import numpy as np

import concourse.bacc as bacc
import concourse.bass as bass
import concourse.mybir as mybir
from concourse.tile import TileContext
from concourse.bass_utils import run_bass_kernel_spmd

F32 = mybir.dt.float32
I32 = mybir.dt.int32
AX = mybir.AxisListType
ALU = mybir.AluOpType
ACTF = mybir.ActivationFunctionType


# ----------------------------------------------------------------------------
# Host-side structure packing (pure index/layout manipulation + reordering)
# ----------------------------------------------------------------------------

def build_structure(edge_index, N, C, W, HALF=32768):
    """Partition nodes across C cores, degree-sort each core's dests into
    windows of 128, and build padded-CSR metadata.

    Returns a dict with per-core packing info plus the shared per-window K
    values (maxed over cores so the SPMD program is identical on all cores).
    """
    NPC = N // C              # owned (real) nodes per core
    NP = W * 128              # padded nodes per core
    src = edge_index[0].astype(np.int64)
    dst = edge_index[1].astype(np.int64)
    E = src.shape[0]

    # append self loops (weight handled separately by caller: w=1)
    loop = np.arange(N, dtype=np.int64)
    src2 = np.concatenate([src, loop])
    dst2 = np.concatenate([dst, loop])
    eid2 = np.arange(E + N, dtype=np.int64)   # index into w2 = [edge_weight, ones]

    owner = dst2 // NPC                        # dest core of each edge
    deg_all = np.bincount(dst2, minlength=N)   # per-dest slot count (incl self)

    # per-core permutation: sort owned dests by degree desc (stable)
    rank = np.empty(N, dtype=np.int64)         # local rank of node on its owner
    for c in range(C):
        lo, hi = c * NPC, (c + 1) * NPC
        order = np.argsort(-deg_all[lo:hi], kind="stable")
        rank[lo + order] = np.arange(NPC)
    node_pos = (np.arange(N) // NPC) * NP + rank      # table row of each node

    cores = []
    KA = np.zeros((C, W), dtype=np.int64)
    KB = np.zeros((C, W), dtype=np.int64)
    for c in range(C):
        sel = owner == c
        e_src = src2[sel]
        e_dst = dst2[sel]
        e_id = eid2[sel]
        dloc = rank[e_dst]                    # local dest rank [0, NPC)
        spos = node_pos[e_src]                # table row of source
        isB = (spos >= HALF).astype(np.int64)
        # sort by (dest rank, phase)
        o = np.lexsort((isB, dloc))
        dloc, spos, isB, e_id = dloc[o], spos[o], isB[o], e_id[o]
        cntA = np.bincount(dloc, weights=1 - isB, minlength=NP).astype(np.int64)
        cntB = np.bincount(dloc, weights=isB, minlength=NP).astype(np.int64)
        starts = np.zeros(NP, dtype=np.int64)
        starts[1:] = np.cumsum(cntA + cntB)[:-1]
        vw = np.arange(NP) // 128
        for w in range(W):
            m = vw == w
            KA[c, w] = cntA[m].max() if m.any() else 0
            KB[c, w] = cntB[m].max() if m.any() else 0
        cores.append(dict(dloc=dloc, spos=spos, isB=isB, eid=e_id,
                          cntA=cntA, cntB=cntB, starts=starts))

    KA = KA.max(axis=0)
    KB = KB.max(axis=0)
    return dict(NPC=NPC, NP=NP, HALF=HALF, C=C, W=W, KA=KA, KB=KB,
                cores=cores, rank=rank, node_pos=node_pos)


def _pad_block(vals, starts, lens, K, fill):
    """[128] ragged segments of `vals` -> padded [128, K] with `fill`."""
    col = np.arange(K)[None, :]
    mask = col < lens[:, None]
    sp = starts[:, None] + col
    sp = np.where(mask, sp, 0)
    out = np.where(mask, vals[sp], fill)
    return out


def pack_core(st, c, w2):
    """Build the int16 index image and f32 weight image for core c.

    Layout per window w: phase A block [128, KA[w]] then phase B block
    [128, KB[w]], concatenated along free dim over all windows.
    idx image: flat k-major wrap -> [128, 8*K] int16 per block.
    """
    W, KA, KB, HALF = st["W"], st["KA"], st["KB"], st["HALF"]
    d = st["cores"][c]
    dloc, spos, isB, eid = d["dloc"], d["spos"], d["isB"], d["eid"]
    cntA, cntB, starts = d["cntA"], d["cntB"], d["starts"]
    wvals = w2[eid]

    idx_cols = []
    w_cols = []
    for w in range(W):
        vs = slice(w * 128, (w + 1) * 128)
        saw = starts[vs]
        caw = cntA[vs]
        cbw = cntB[vs]
        for K, stt, ln, off in ((int(KA[w]), saw, caw, 0),
                                (int(KB[w]), saw + caw, cbw, HALF)):
            if K == 0:
                continue
            pi = _pad_block(spos, stt, ln, K, off).astype(np.int64) - off
            pw = _pad_block(wvals, stt, ln, K, 0.0)
            assert pi.min() >= 0
            idx_cols.append(pi.astype(np.int32))          # [128, K]
            w_cols.append(pw.astype(np.float32))          # [128, K]
    idx_img = np.concatenate(idx_cols, axis=1)
    w_img = np.concatenate(w_cols, axis=1)
    return idx_img, w_img


# ----------------------------------------------------------------------------
# Bass program
# ----------------------------------------------------------------------------

def build_program(st, L, D=128):
    W = st["W"]
    NP = st["NP"]
    HALF = st["HALF"]
    C = st["C"]
    KA, KB = st["KA"], st["KB"]
    KT = [int(KA[w] + KB[w]) for w in range(W)]
    KCOLS = int(sum(KT))
    IDXCOLS = KCOLS
    NT = NP * C                     # table rows

    nc = bacc.Bacc("TRN2", target_bir_lowering=False, debug=True)

    x_in = nc.dram_tensor("x_shard", [NP, D], F32, kind="ExternalInput")
    idx_in = nc.dram_tensor("idx_img", [128, IDXCOLS], I32, kind="ExternalInput")
    w_in = nc.dram_tensor("w_img", [128, KCOLS], F32, kind="ExternalInput")
    wst_in = nc.dram_tensor("wst", [L, D, D], F32, kind="ExternalInput")
    bias_in = nc.dram_tensor("bias_b", [L, D, D], F32, kind="ExternalInput")
    gam_in = nc.dram_tensor("gamma_b", [L, D, D], F32, kind="ExternalInput")
    bet_in = nc.dram_tensor("beta_b", [L, D, D], F32, kind="ExternalInput")
    id_in = nc.dram_tensor("ident", [D, D], F32, kind="ExternalInput")
    out_t = nc.dram_tensor("out_shard", [NP, D], F32, kind="ExternalOutput")

    with TileContext(nc) as tc:
        with (
            tc.tile_pool(name="persist", bufs=1) as pp,
            tc.tile_pool(name="gath", bufs=3) as gp,
            tc.tile_pool(name="work", bufs=3) as wk,
            tc.tile_pool(name="tiny", bufs=4) as tn,
            tc.tile_pool(name="psum", bufs=2, space="PSUM") as ps,
            tc.tile_pool(name="dram", bufs=1, space="DRAM") as dr,
        ):
            # ---- persistent SBUF state ----
            h = pp.tile([128, W, D], F32, tag="h")
            idx = pp.tile([128, IDXCOLS], I32, tag="idx")
            wn = pp.tile([128, KCOLS], F32, tag="wn")      # weights -> norm
            wst = pp.tile([128, L * D], F32, tag="wst")
            biasb = pp.tile([128, L * D], F32, tag="biasb")
            gamb = pp.tile([128, L * D], F32, tag="gamb")
            betb = pp.tile([128, L * D], F32, tag="betb")
            ident = pp.tile([128, D], F32, tag="ident")
            dinv = pp.tile([128, W], F32, tag="dinv")

            nc.sync.dma_start(out=h[:, :, :],
                              in_=x_in[:].rearrange("(w p) f -> p w f", p=128))
            nc.sync.dma_start(out=idx[:, :], in_=idx_in[:, :])
            nc.sync.dma_start(out=wn[:, :], in_=w_in[:, :])
            for l in range(L):
                for dst_t, src_t in ((wst, wst_in), (biasb, bias_in),
                                     (gamb, gam_in), (betb, bet_in)):
                    nc.sync.dma_start(out=dst_t[:, l * D:(l + 1) * D],
                                      in_=src_t[l, :, :])
            nc.sync.dma_start(out=ident[:, :], in_=id_in[:, :])

            # ---- degree -> dinv (once; includes self-loop weights) ----
            deg = tn.tile([128, W], F32, tag="deg")
            off = 0
            for w in range(W):
                blk = wn[:, off:off + KT[w]]
                nc.vector.tensor_reduce(deg[:, w:w + 1], blk, AX.X, ALU.add)
                off += KT[w]
            rdeg = tn.tile([128, W], F32, tag="rdeg")
            nc.vector.reciprocal(rdeg[:, :], deg[:, :])
            nc.scalar.sqrt(dinv[:, :], rdeg[:, :])
            # norm = w * dinv[dest]  (in place on wn)
            off = 0
            for w in range(W):
                nc.vector.tensor_scalar_mul(
                    wn[:, off:off + KT[w]], wn[:, off:off + KT[w]],
                    dinv[:, w:w + 1])
                off += KT[w]

            # ---- per-layer DRAM tables (double buffered across layers) ----
            tables = [dr.tile([NT, D], F32, name=f"table{i}", tag=f"table{i}") for i in range(2)]
            xw_own = [dr.tile([NP, D], F32, name=f"xwown{i}", tag=f"xwown{i}") for i in range(2)]

            for li in range(L):
                tab = tables[li % 2]
                own = xw_own[li % 2]
                wst_l = wst[:, li * D:(li + 1) * D]
                # -- build own table shard: T = dinv * (h @ Ws^T) --
                for w in range(W):
                    hT = ps.tile([128, D], F32, tag="hT")
                    nc.tensor.transpose(hT[:, :], h[:, w, :], ident[:, :])
                    hTs = wk.tile([128, D], F32, tag="hTs")
                    nc.scalar.activation(hTs[:, :], hT[:, :], ACTF.Copy)
                    mm = ps.tile([128, D], F32, tag="mm")
                    nc.tensor.matmul(mm[:, :], hTs[:, :], wst_l)
                    xw = wk.tile([128, D], F32, tag="xw")
                    nc.scalar.activation(xw[:, :], mm[:, :], ACTF.Copy,
                                         scale=dinv[:, w:w + 1])
                    nc.sync.dma_start(out=own[w * 128:(w + 1) * 128, :],
                                      in_=xw[:, :])
                nc.gpsimd.collective_compute(
                    "AllGather", ALU.bypass,
                    replica_groups=[list(range(C))],
                    ins=[own[:].opt()], outs=[tab[:].opt()])

                # -- aggregate into owned dests --
                off_k = 0
                off_i = 0
                for w in range(W):
                    ka, kb = int(KA[w]), int(KB[w])
                    kt = ka + kb
                    g = gp.tile([128, kt, D], F32, tag="g")
                    for k in range(kt):
                        nc.gpsimd.indirect_dma_start(
                            out=g[:, k, :], out_offset=None,
                            in_=tab[:, :],
                            in_offset=bass.IndirectOffsetOnAxis(
                                ap=idx[:, off_i + k:off_i + k + 1], axis=0))
                    # scale by per-(dest,k) norm, broadcast over features
                    nw = wn[:, off_k:off_k + kt].unsqueeze(2)
                    nc.vector.tensor_tensor(
                        g[:, :, :], g[:, :, :],
                        nw.broadcast_to([128, kt, D]), ALU.mult)
                    # reduce over k (strided innermost)
                    agg = wk.tile([128, D], F32, tag="agg")
                    nc.vector.tensor_reduce(
                        agg[:, :], g[:, :, :].transpose([0, 2, 1]),
                        AX.X, ALU.add)
                    # x0 = agg*dinv + bias
                    x0 = wk.tile([128, D], F32, tag="x0")
                    nc.vector.tensor_scalar_mul(x0[:, :], agg[:, :],
                                                dinv[:, w:w + 1])
                    nc.vector.tensor_add(x0[:, :], x0[:, :],
                                         biasb[:, li * D:(li + 1) * D])
                    # layernorm
                    sx = tn.tile([128, 1], F32, tag="sx")
                    nc.vector.tensor_reduce(sx[:, :], x0[:, :], AX.X, ALU.add)
                    sq = tn.tile([128, 1], F32, tag="sq")
                    sqs = wk.tile([128, D], F32, tag="sqs")
                    nc.scalar.activation(sqs[:, :], x0[:, :], ACTF.Square,
                                         accum_out=sq[:, :])
                    mu = tn.tile([128, 1], F32, tag="mu")
                    nc.vector.tensor_scalar_mul(mu[:, :], sx[:, :], 1.0 / D)
                    ms = tn.tile([128, 1], F32, tag="ms")
                    nc.vector.tensor_scalar(ms[:, :], sq[:, :], 1.0 / D,
                                            1e-5, ALU.mult, ALU.add)
                    mu2 = tn.tile([128, 1], F32, tag="mu2")
                    nc.vector.tensor_mul(mu2[:, :], mu[:, :], mu[:, :])
                    var = tn.tile([128, 1], F32, tag="var")
                    nc.vector.tensor_sub(var[:, :], ms[:, :], mu2[:, :])
                    rv = tn.tile([128, 1], F32, tag="rv")
                    nc.vector.reciprocal(rv[:, :], var[:, :])
                    rstd = tn.tile([128, 1], F32, tag="rstd")
                    nc.scalar.sqrt(rstd[:, :], rv[:, :])
                    nmr = tn.tile([128, 1], F32, tag="nmr")
                    nc.vector.tensor_mul(nmr[:, :], mu[:, :], rstd[:, :])
                    t = wk.tile([128, D], F32, tag="t")
                    nc.vector.tensor_scalar(t[:, :], x0[:, :], rstd[:, :],
                                            nmr[:, :], ALU.mult, ALU.subtract)
                    nc.vector.tensor_mul(t[:, :], t[:, :],
                                         gamb[:, li * D:(li + 1) * D])
                    nc.vector.tensor_add(t[:, :], t[:, :],
                                         betb[:, li * D:(li + 1) * D])
                    if li < L - 1:
                        nc.scalar.activation(t[:, :], t[:, :], ACTF.Relu)
                    nc.vector.tensor_add(h[:, w, :], t[:, :], h[:, w, :])
                    off_k += kt
                    off_i += kt

            nc.sync.dma_start(out=out_t[:].rearrange("(w p) f -> p w f", p=128),
                              in_=h[:, :, :])

    nc.compile()
    return nc


# ----------------------------------------------------------------------------
# Full kernel entry
# ----------------------------------------------------------------------------

def _kernel_impl(x, edge_index, edge_weight, Ws, bs, gammas, betas,
                 C=8, W=49, HALF=1 << 60, trace=False):
    N, D = x.shape
    L = Ws.shape[0]
    st = build_structure(edge_index, N, C, W, HALF)
    NP, NPC = st["NP"], st["NPC"]

    w2 = np.concatenate([np.asarray(edge_weight, dtype=np.float32),
                         np.ones(N, dtype=np.float32)])

    ident = np.eye(D, dtype=np.float32)
    wst = np.ascontiguousarray(np.transpose(np.asarray(Ws), (0, 2, 1)))
    bias_b = np.ascontiguousarray(
        np.broadcast_to(np.asarray(bs)[:, None, :], (L, D, D))).astype(np.float32)
    gam_b = np.ascontiguousarray(
        np.broadcast_to(np.asarray(gammas)[:, None, :], (L, D, D))).astype(np.float32)
    bet_b = np.ascontiguousarray(
        np.broadcast_to(np.asarray(betas)[:, None, :], (L, D, D))).astype(np.float32)

    in_maps = []
    for c in range(C):
        idx_img, w_img = pack_core(st, c, w2)
        xs = np.zeros((NP, D), dtype=np.float32)
        lo = c * NPC
        xs[st["rank"][lo:lo + NPC]] = np.asarray(x[lo:lo + NPC], dtype=np.float32)
        in_maps.append(dict(x_shard=xs, idx_img=idx_img, w_img=w_img,
                            wst=wst, bias_b=bias_b, gamma_b=gam_b,
                            beta_b=bet_b, ident=ident))

    nc = build_program(st, L, D)
    res = run_bass_kernel_spmd(nc, in_maps, list(range(C)), trace=trace)

    out = np.empty((N, D), dtype=np.float32)
    for c in range(C):
        lo = c * NPC
        sh = res.results[c]["out_shard"]
        out[lo:lo + NPC] = sh[st["rank"][lo:lo + NPC]]
    return out, res


def kernel(x, edge_index, edge_weight, Ws, bs, gammas, betas):
    out, _ = _kernel_impl(np.asarray(x), np.asarray(edge_index),
                          np.asarray(edge_weight), np.asarray(Ws),
                          np.asarray(bs), np.asarray(gammas), np.asarray(betas))
    return out



# revision 6
# speedup vs baseline: 1.3976x; 1.3976x over previous
"""GCN encoder (3-layer GCNConv + LayerNorm + ReLU + residual) on 8 TRN2
NeuronCores via Bass/Tile.

Sharding: nodes are partitioned across cores (graph parallel). Per layer each
core computes its own dinv-scaled xw shard (bf16), AllGathers the full table
to Shared DRAM, then aggregates its in-edges with batched `dma_gather` row
gathers (int16 indices, phase A/B around row 32768) and a PE matmul against
host-built one-hot S blocks ([128 edges, 128 dests] bf16 carrying the edge
weight), accumulating each dest window in PSUM. dinv[dst] + conv bias +
LayerNorm + ReLU + residual are applied per window as before.
"""

import numpy as np
import ml_dtypes

import concourse.bacc as bacc
import concourse.bass as bass
import concourse.mybir as mybir
from concourse.tile import TileContext
from concourse.bass_utils import run_bass_kernel_spmd
from concourse.library_config import mlp as mlp_library

F32 = mybir.dt.float32
BF16 = mybir.dt.bfloat16
I16 = mybir.dt.int16
AX = mybir.AxisListType
ALU = mybir.AluOpType
ACTF = mybir.ActivationFunctionType

BF16NP = ml_dtypes.bfloat16


# ----------------------------------------------------------------------------
# Host-side structure packing (pure index/layout manipulation + reordering)
# ----------------------------------------------------------------------------

def build_structure(edge_index, edge_weight, N, C, W, HALF=32768):
    """Partition nodes across C cores, degree-sort each core's dests into
    windows of 128, split each window's in-edges into phase A (table row <
    HALF) / phase B, pad both to 128-edge blocks (block counts maxed over
    cores so the SPMD program is identical), and emit per-core images:

      idx_img  [128, TOT/16] int16 : dma_gather indices, 16-wrapped + x8 replicated
      s_img    [128, TOT]    bf16  : one-hot-times-w S blocks (lhsT layout)
      wdeg_img [128, KCOLS]  f32   : per-dest padded edge weights (deg reduce)
    """
    NPC = N // C
    NP = W * 128
    src = np.asarray(edge_index[0], dtype=np.int64)
    dst = np.asarray(edge_index[1], dtype=np.int64)
    E = src.shape[0]

    loop = np.arange(N, dtype=np.int64)
    src2 = np.concatenate([src, loop])
    dst2 = np.concatenate([dst, loop])
    w2 = np.concatenate([np.asarray(edge_weight, dtype=np.float32),
                         np.ones(N, dtype=np.float32)])

    deg_all = np.bincount(dst2, minlength=N)

    # per-core permutation: sort owned dests by degree desc (stable)
    rank = np.empty(N, dtype=np.int64)
    for c in range(C):
        lo, hi = c * NPC, (c + 1) * NPC
        order = np.argsort(-deg_all[lo:hi], kind="stable")
        rank[lo + order] = np.arange(NPC)
    node_pos = (np.arange(N) // NPC) * NP + rank      # table row of each node

    owner = dst2 // NPC
    spos = node_pos[src2]
    dloc = rank[dst2]

    per_core = []
    cntAB = np.zeros((C, W, 2), dtype=np.int64)
    for c in range(C):
        sel = owner == c
        e_spos = spos[sel]
        e_dloc = dloc[sel]
        e_w = w2[sel]
        win = e_dloc // 128
        isB = (e_spos >= HALF).astype(np.int64)
        o = np.lexsort((isB, win))
        e_spos, e_dloc, e_w, win, isB = (
            e_spos[o], e_dloc[o], e_w[o], win[o], isB[o])
        cnt = np.zeros((W, 2), dtype=np.int64)
        np.add.at(cnt, (win, isB), 1)
        cntAB[c] = cnt
        per_core.append(dict(spos=e_spos, dloc=e_dloc, w=e_w, win=win,
                             isB=isB, cnt=cnt))

    nblk = (np.ceil(cntAB / 128.0).astype(np.int64)).max(axis=0)  # [W, 2]
    # per-dest K values for the deg image (padded-K layout per window)
    KDEG = np.zeros(W, dtype=np.int64)
    deg_pad = np.zeros((C, NP), dtype=np.int64)
    for c in range(C):
        lo = c * NPC
        deg_pad[c, rank[lo:lo + NPC]] = deg_all[lo:lo + NPC]
    for w in range(W):
        KDEG[w] = deg_pad[:, w * 128:(w + 1) * 128].max()

    return dict(NPC=NPC, NP=NP, NT=NP * C, HALF=HALF, C=C, W=W,
                nblk=nblk, KDEG=KDEG, per_core=per_core, rank=rank,
                node_pos=node_pos, w2=w2)


def pack_core(st, c):
    """Build idx_img (int16), s_img (bf16) and wdeg_img (f32) for core c."""
    W, HALF = st["W"], st["HALF"]
    nblk = st["nblk"]
    d = st["per_core"][c]
    spos, dloc, wv, win, isB = d["spos"], d["dloc"], d["w"], d["win"], d["isB"]
    cnt = d["cnt"]

    NBLK = int(nblk.sum())
    TOT = NBLK * 128

    idx_flat = np.zeros(TOT, dtype=np.int16)
    dl_flat = np.zeros(TOT, dtype=np.int64)
    w_flat = np.zeros(TOT, dtype=np.float32)

    # edges are sorted by (win, phase); compute each edge's padded position
    base = 0          # padded base position of current (win, phase) group
    src_off = 0       # offset into the sorted edge arrays
    for w in range(W):
        for ph in range(2):
            n = int(cnt[w, ph])
            npad = int(nblk[w, ph]) * 128
            if n:
                sl = slice(src_off, src_off + n)
                ii = np.arange(base, base + n)
                rel = spos[sl] - (HALF if ph else 0)
                assert rel.min() >= 0 and rel.max() < 32768
                idx_flat[ii] = rel.astype(np.int16)
                dl_flat[ii] = dloc[sl] % 128
                w_flat[ii] = wv[sl]
            src_off += n
            base += npad
    assert base == TOT

    # idx image: j -> partition j%16, col j//16, replicated x8
    idx_img = np.tile(idx_flat.reshape(TOT // 16, 16).T, (8, 1))
    idx_img = np.ascontiguousarray(idx_img, dtype=np.int16)

    # S image: [128, TOT]; S[j%128, (j//128)*128 + dloc_j] = w_j
    s_img = np.zeros((128, TOT), dtype=np.float32)
    j = np.arange(TOT)
    s_img[j % 128, (j // 128) * 128 + dl_flat] = w_flat
    s_img = s_img.astype(BF16NP)

    # wdeg image: per window, [128, KDEG[w]] of the dest's edge weights
    KDEG = st["KDEG"]
    NP = st["NP"]
    cols = []
    dl_all = dloc  # local dest rank in [0, NP)
    order = np.argsort(dl_all, kind="stable")
    dl_s = dl_all[order]
    w_s = wv[order]
    starts = np.searchsorted(dl_s, np.arange(NP))
    ends = np.searchsorted(dl_s, np.arange(NP) + 1)
    for w in range(W):
        K = int(KDEG[w])
        if K == 0:
            continue
        blk = np.zeros((128, K), dtype=np.float32)
        for p in range(128):
            dd = w * 128 + p
            s, e = starts[dd], ends[dd]
            blk[p, : e - s] = w_s[s:e]
        cols.append(blk)
    wdeg_img = np.concatenate(cols, axis=1)
    return idx_img, s_img, wdeg_img


# ----------------------------------------------------------------------------
# Bass program
# ----------------------------------------------------------------------------

def build_program(st, L, D=128):
    W = st["W"]
    NP = st["NP"]
    NT = st["NT"]
    HALF = st["HALF"]
    C = st["C"]
    nblk = st["nblk"]
    KDEG = st["KDEG"]
    NBLK = int(nblk.sum())
    TOT = NBLK * 128
    IDXW = TOT // 16
    KCOLS = int(KDEG.sum())

    nc = bacc.Bacc("TRN2", target_bir_lowering=False, debug=True)

    x_in = nc.dram_tensor("x_shard", [NP, D], F32, kind="ExternalInput")
    idx_in = nc.dram_tensor("idx_img", [128, IDXW], I16, kind="ExternalInput")
    s_in = nc.dram_tensor("s_img", [128, TOT], BF16, kind="ExternalInput")
    wdeg_in = nc.dram_tensor("wdeg_img", [128, KCOLS], F32, kind="ExternalInput")
    wst_in = nc.dram_tensor("wst", [L, D, D], F32, kind="ExternalInput")
    bias_in = nc.dram_tensor("bias_b", [L, D, D], F32, kind="ExternalInput")
    gam_in = nc.dram_tensor("gamma_b", [L, D, D], F32, kind="ExternalInput")
    bet_in = nc.dram_tensor("beta_b", [L, D, D], F32, kind="ExternalInput")
    id_in = nc.dram_tensor("ident", [D, D], F32, kind="ExternalInput")
    out_t = nc.dram_tensor("out_shard", [NP, D], F32, kind="ExternalOutput")

    with TileContext(nc) as tc:
        with (
            tc.tile_pool(name="persist", bufs=1) as pp,
            tc.tile_pool(name="gath", bufs=3) as gp,
            tc.tile_pool(name="smat", bufs=3) as sp,
            tc.tile_pool(name="work", bufs=3) as wk,
            tc.tile_pool(name="tiny", bufs=4) as tn,
            tc.tile_pool(name="psum", bufs=2, space="PSUM") as ps,
            tc.tile_pool(name="psagg", bufs=4, space="PSUM") as pagg,
            tc.tile_pool(name="dram", bufs=1, space="DRAM") as dr,
        ):
            # ---- persistent SBUF state ----
            h = pp.tile([128, W, D], F32, tag="h")
            idx = pp.tile([128, IDXW], I16, tag="idx")
            wdeg = pp.tile([128, KCOLS], F32, tag="wdeg")
            wst = pp.tile([128, L * D], F32, tag="wst")
            biasb = pp.tile([128, L * D], F32, tag="biasb")
            gamb = pp.tile([128, L * D], F32, tag="gamb")
            betb = pp.tile([128, L * D], F32, tag="betb")
            ident = pp.tile([128, D], F32, tag="ident")
            dinv = pp.tile([128, W], F32, tag="dinv")

            nc.sync.dma_start(out=h[:, :, :],
                              in_=x_in[:].rearrange("(w p) f -> p w f", p=128))
            nc.sync.dma_start(out=idx[:, :], in_=idx_in[:, :])
            nc.sync.dma_start(out=wdeg[:, :], in_=wdeg_in[:, :])
            for l in range(L):
                for dst_t, src_t in ((wst, wst_in), (biasb, bias_in),
                                     (gamb, gam_in), (betb, bet_in)):
                    nc.sync.dma_start(out=dst_t[:, l * D:(l + 1) * D],
                                      in_=src_t[l, :, :])
            nc.sync.dma_start(out=ident[:, :], in_=id_in[:, :])

            nc.gpsimd.load_library(mlp_library)

            # ---- degree -> dinv (once; includes self-loop weights) ----
            deg = tn.tile([128, W], F32, tag="deg")
            off = 0
            for w in range(W):
                K = int(KDEG[w])
                nc.vector.tensor_reduce(deg[:, w:w + 1], wdeg[:, off:off + K],
                                        AX.X, ALU.add)
                off += K
            rdeg = tn.tile([128, W], F32, tag="rdeg")
            nc.vector.reciprocal(rdeg[:, :], deg[:, :])
            nc.scalar.sqrt(dinv[:, :], rdeg[:, :])

            # ---- per-layer DRAM tables (double buffered across layers) ----
            tables = [dr.tile([NT, D], BF16, name=f"table{i}", tag=f"table{i}",
                              addr_space="Local") for i in range(L)]
            xw_own = [dr.tile([NP, D], BF16, name=f"xwown{i}", tag=f"xwown{i}")
                      for i in range(2)]

            for li in range(L):
                tab = tables[li]
                own = xw_own[li % 2]
                wst_l = wst[:, li * D:(li + 1) * D]
                # -- build own table shard: T = bf16(dinv * (h @ Ws^T)) --
                for w in range(W):
                    hT = ps.tile([128, D], F32, tag="hT")
                    nc.tensor.transpose(hT[:, :], h[:, w, :], ident[:, :])
                    hTs = wk.tile([128, D], F32, tag="hTs")
                    nc.scalar.activation(hTs[:, :], hT[:, :], ACTF.Copy)
                    mm = ps.tile([128, D], F32, tag="mm")
                    nc.tensor.matmul(mm[:, :], hTs[:, :], wst_l)
                    xwb = wk.tile([128, D], BF16, tag="xwb")
                    nc.scalar.activation(xwb[:, :], mm[:, :], ACTF.Copy,
                                         scale=dinv[:, w:w + 1])
                    nc.sync.dma_start(out=own[w * 128:(w + 1) * 128, :],
                                      in_=xwb[:, :])
                nc.gpsimd.collective_compute(
                    "AllGather", ALU.bypass,
                    replica_groups=[list(range(C))],
                    ins=[own[:].opt()], outs=[tab[:].opt()])

                # -- aggregate into owned dests --
                pos = 0
                for w in range(W):
                    nA, nB = int(nblk[w, 0]), int(nblk[w, 1])
                    nT = nA + nB
                    g = gp.tile([128, nT, D], BF16, tag="g")
                    if nA:
                        nc.gpsimd.dma_gather(
                            g[:, 0:nA, :], tab[:, :],
                            idx[:, pos * 8:(pos + nA) * 8],
                            nA * 128, nA * 128, D, single_packet=False)
                    if nB:
                        nc.gpsimd.dma_gather(
                            g[:, nA:nT, :], tab[HALF:, :],
                            idx[:, (pos + nA) * 8:(pos + nT) * 8],
                            nB * 128, nB * 128, D, single_packet=False)
                    s_t = sp.tile([128, nT, 128], BF16, tag="s_t")
                    nc.sync.dma_start(
                        out=s_t[:, :, :],
                        in_=s_in[:, pos * 128:(pos + nT) * 128])
                    agg = pagg.tile([128, D], F32, tag="agg")
                    for b in range(nT):
                        nc.tensor.matmul(agg[:, :], s_t[:, b, :], g[:, b, :],
                                         start=(b == 0), stop=(b == nT - 1))
                    # x0 = agg*dinv + bias
                    x0 = wk.tile([128, D], F32, tag="x0")
                    nc.scalar.activation(x0[:, :], agg[:, :], ACTF.Copy,
                                         scale=dinv[:, w:w + 1])
                    nc.vector.tensor_add(x0[:, :], x0[:, :],
                                         biasb[:, li * D:(li + 1) * D])
                    # layernorm
                    sx = tn.tile([128, 1], F32, tag="sx")
                    nc.vector.tensor_reduce(sx[:, :], x0[:, :], AX.X, ALU.add)
                    sq = tn.tile([128, 1], F32, tag="sq")
                    sqs = wk.tile([128, D], F32, tag="sqs")
                    nc.scalar.activation(sqs[:, :], x0[:, :], ACTF.Square,
                                         accum_out=sq[:, :])
                    mu = tn.tile([128, 1], F32, tag="mu")
                    nc.vector.tensor_scalar_mul(mu[:, :], sx[:, :], 1.0 / D)
                    ms = tn.tile([128, 1], F32, tag="ms")
                    nc.vector.tensor_scalar(ms[:, :], sq[:, :], 1.0 / D,
                                            1e-5, ALU.mult, ALU.add)
                    mu2 = tn.tile([128, 1], F32, tag="mu2")
                    nc.vector.tensor_mul(mu2[:, :], mu[:, :], mu[:, :])
                    var = tn.tile([128, 1], F32, tag="var")
                    nc.vector.tensor_sub(var[:, :], ms[:, :], mu2[:, :])
                    rv = tn.tile([128, 1], F32, tag="rv")
                    nc.vector.reciprocal(rv[:, :], var[:, :])
                    rstd = tn.tile([128, 1], F32, tag="rstd")
                    nc.scalar.sqrt(rstd[:, :], rv[:, :])
                    nmr = tn.tile([128, 1], F32, tag="nmr")
                    nc.vector.tensor_mul(nmr[:, :], mu[:, :], rstd[:, :])
                    t = wk.tile([128, D], F32, tag="t")
                    nc.vector.tensor_scalar(t[:, :], x0[:, :], rstd[:, :],
                                            nmr[:, :], ALU.mult, ALU.subtract)
                    nc.vector.tensor_mul(t[:, :], t[:, :],
                                         gamb[:, li * D:(li + 1) * D])
                    nc.vector.tensor_add(t[:, :], t[:, :],
                                         betb[:, li * D:(li + 1) * D])
                    if li < L - 1:
                        nc.scalar.activation(t[:, :], t[:, :], ACTF.Relu)
                    nc.vector.tensor_add(h[:, w, :], t[:, :], h[:, w, :])
                    pos += nT

            nc.sync.dma_start(out=out_t[:].rearrange("(w p) f -> p w f", p=128),
                              in_=h[:, :, :])

    nc.compile()
    return nc


# ----------------------------------------------------------------------------
# Full kernel entry
# ----------------------------------------------------------------------------

def _kernel_impl(x, edge_index, edge_weight, Ws, bs, gammas, betas,
                 C=8, W=49, HALF=32768, trace=False):
    N, D = x.shape
    L = Ws.shape[0]
    st = build_structure(edge_index, edge_weight, N, C, W, HALF)
    NP, NPC = st["NP"], st["NPC"]

    ident = np.eye(D, dtype=np.float32)
    wst = np.ascontiguousarray(np.transpose(np.asarray(Ws), (0, 2, 1))).astype(np.float32)
    bias_b = np.ascontiguousarray(
        np.broadcast_to(np.asarray(bs)[:, None, :], (L, D, D))).astype(np.float32)
    gam_b = np.ascontiguousarray(
        np.broadcast_to(np.asarray(gammas)[:, None, :], (L, D, D))).astype(np.float32)
    bet_b = np.ascontiguousarray(
        np.broadcast_to(np.asarray(betas)[:, None, :], (L, D, D))).astype(np.float32)

    in_maps = []
    for c in range(C):
        idx_img, s_img, wdeg_img = pack_core(st, c)
        xs = np.zeros((NP, D), dtype=np.float32)
        lo = c * NPC
        xs[st["rank"][lo:lo + NPC]] = np.asarray(x[lo:lo + NPC], dtype=np.float32)
        in_maps.append(dict(x_shard=xs, idx_img=idx_img, s_img=s_img,
                            wdeg_img=wdeg_img, wst=wst, bias_b=bias_b,
                            gamma_b=gam_b, beta_b=bet_b, ident=ident))

    nc = build_program(st, L, D)
    res = run_bass_kernel_spmd(nc, in_maps, list(range(C)), trace=trace)

    out = np.empty((N, D), dtype=np.float32)
    for c in range(C):
        lo = c * NPC
        sh = res.results[c]["out_shard"]
        out[lo:lo + NPC] = sh[st["rank"][lo:lo + NPC]]
    return out, res


def kernel(x, edge_index, edge_weight, Ws, bs, gammas, betas):
    out, _ = _kernel_impl(np.asarray(x), np.asarray(edge_index),
                          np.asarray(edge_weight), np.asarray(Ws),
                          np.asarray(bs), np.asarray(gammas), np.asarray(betas))
    return out


# revision 8
# speedup vs baseline: 2.4850x; 1.7780x over previous
"""GCN encoder (3-layer GCNConv + LayerNorm + ReLU + residual) on 8 TRN2
NeuronCores via Bass/Tile.

Sharding: nodes are partitioned across cores (graph parallel). Per layer each
core computes its own dinv-scaled xw shard (bf16), AllGathers the full table
to Shared DRAM, then aggregates its in-edges with batched `dma_gather` row
gathers (int16 indices, phase A/B around row 32768) and a PE matmul against
host-built one-hot S blocks ([128 edges, 128 dests] bf16 carrying the edge
weight), accumulating each dest window in PSUM. dinv[dst] + conv bias +
LayerNorm + ReLU + residual are applied per window as before.
"""

import numpy as np
import ml_dtypes

import concourse.bacc as bacc
import concourse.bass as bass
import concourse.mybir as mybir
from concourse.tile import TileContext
from concourse.bass_utils import run_bass_kernel_spmd
from concourse.library_config import mlp as mlp_library

F32 = mybir.dt.float32
BF16 = mybir.dt.bfloat16
I16 = mybir.dt.int16
AX = mybir.AxisListType
ALU = mybir.AluOpType
ACTF = mybir.ActivationFunctionType

BF16NP = ml_dtypes.bfloat16


# ----------------------------------------------------------------------------
# Host-side structure packing (pure index/layout manipulation + reordering)
# ----------------------------------------------------------------------------

def build_structure(edge_index, edge_weight, N, C, W, HALF=32768):
    """Partition nodes across C cores, degree-sort each core's dests into
    windows of 128, split each window's in-edges into phase A (table row <
    HALF) / phase B, pad both to 128-edge blocks (block counts maxed over
    cores so the SPMD program is identical), and emit per-core images:

      idx_img  [128, TOT/16] int16 : dma_gather indices, 16-wrapped + x8 replicated
      s_img    [128, TOT]    bf16  : one-hot-times-w S blocks (lhsT layout)
      wdeg_img [128, KCOLS]  f32   : per-dest padded edge weights (deg reduce)
    """
    NPC = N // C
    NP = W * 128
    src = np.asarray(edge_index[0], dtype=np.int64)
    dst = np.asarray(edge_index[1], dtype=np.int64)
    E = src.shape[0]

    loop = np.arange(N, dtype=np.int64)
    src2 = np.concatenate([src, loop])
    dst2 = np.concatenate([dst, loop])
    w2 = np.concatenate([np.asarray(edge_weight, dtype=np.float32),
                         np.ones(N, dtype=np.float32)])

    deg_all = np.bincount(dst2, minlength=N)

    # per-core permutation: sort owned dests by degree desc (stable)
    rank = np.empty(N, dtype=np.int64)
    for c in range(C):
        lo, hi = c * NPC, (c + 1) * NPC
        order = np.argsort(-deg_all[lo:hi], kind="stable")
        rank[lo + order] = np.arange(NPC)
    node_pos = (np.arange(N) // NPC) * NP + rank      # table row of each node

    owner = dst2 // NPC
    spos = node_pos[src2]
    dloc = rank[dst2]

    per_core = []
    cntAB = np.zeros((C, W, 2), dtype=np.int64)
    for c in range(C):
        sel = owner == c
        e_spos = spos[sel]
        e_dloc = dloc[sel]
        e_w = w2[sel]
        win = e_dloc // 128
        isB = (e_spos >= HALF).astype(np.int64)
        o = np.lexsort((isB, win))
        e_spos, e_dloc, e_w, win, isB = (
            e_spos[o], e_dloc[o], e_w[o], win[o], isB[o])
        cnt = np.zeros((W, 2), dtype=np.int64)
        np.add.at(cnt, (win, isB), 1)
        cntAB[c] = cnt
        per_core.append(dict(spos=e_spos, dloc=e_dloc, w=e_w, win=win,
                             isB=isB, cnt=cnt))

    nblk = (np.ceil(cntAB / 128.0).astype(np.int64)).max(axis=0)  # [W, 2]
    # per-dest K values for the deg image (padded-K layout per window)
    KDEG = np.zeros(W, dtype=np.int64)
    deg_pad = np.zeros((C, NP), dtype=np.int64)
    for c in range(C):
        lo = c * NPC
        deg_pad[c, rank[lo:lo + NPC]] = deg_all[lo:lo + NPC]
    for w in range(W):
        KDEG[w] = deg_pad[:, w * 128:(w + 1) * 128].max()

    return dict(NPC=NPC, NP=NP, NT=NP * C, HALF=HALF, C=C, W=W,
                nblk=nblk, KDEG=KDEG, per_core=per_core, rank=rank,
                node_pos=node_pos, w2=w2)


def pack_core(st, c):
    """Build idx_img (int16), s_img (bf16) and wdeg_img (f32) for core c."""
    W, HALF = st["W"], st["HALF"]
    nblk = st["nblk"]
    d = st["per_core"][c]
    spos, dloc, wv, win, isB = d["spos"], d["dloc"], d["w"], d["win"], d["isB"]
    cnt = d["cnt"]

    NBLK = int(nblk.sum())
    TOT = NBLK * 128

    idx_flat = np.zeros(TOT, dtype=np.int16)
    dl_flat = np.zeros(TOT, dtype=np.int64)
    w_flat = np.zeros(TOT, dtype=np.float32)

    # edges are sorted by (win, phase); compute each edge's padded position
    base = 0          # padded base position of current (win, phase) group
    src_off = 0       # offset into the sorted edge arrays
    for w in range(W):
        for ph in range(2):
            n = int(cnt[w, ph])
            npad = int(nblk[w, ph]) * 128
            if n:
                sl = slice(src_off, src_off + n)
                ii = np.arange(base, base + n)
                rel = spos[sl] - (HALF if ph else 0)
                assert rel.min() >= 0 and rel.max() < 32768
                idx_flat[ii] = rel.astype(np.int16)
                dl_flat[ii] = dloc[sl] % 128
                w_flat[ii] = wv[sl]
            src_off += n
            base += npad
    assert base == TOT

    # idx image: j -> partition j%16, col j//16, replicated x8
    idx_img = np.tile(idx_flat.reshape(TOT // 16, 16).T, (8, 1))
    idx_img = np.ascontiguousarray(idx_img, dtype=np.int16)

    # S image: [128, TOT]; S[j%128, (j//128)*128 + dloc_j] = w_j
    s_img = np.zeros((128, TOT), dtype=np.float32)
    j = np.arange(TOT)
    s_img[j % 128, (j // 128) * 128 + dl_flat] = w_flat
    s_img = s_img.astype(BF16NP)

    # wdeg image: per window, [128, KDEG[w]] of the dest's edge weights
    KDEG = st["KDEG"]
    NP = st["NP"]
    cols = []
    dl_all = dloc  # local dest rank in [0, NP)
    order = np.argsort(dl_all, kind="stable")
    dl_s = dl_all[order]
    w_s = wv[order]
    starts = np.searchsorted(dl_s, np.arange(NP))
    ends = np.searchsorted(dl_s, np.arange(NP) + 1)
    for w in range(W):
        K = int(KDEG[w])
        if K == 0:
            continue
        blk = np.zeros((128, K), dtype=np.float32)
        for p in range(128):
            dd = w * 128 + p
            s, e = starts[dd], ends[dd]
            blk[p, : e - s] = w_s[s:e]
        cols.append(blk)
    wdeg_img = np.concatenate(cols, axis=1)
    return idx_img, s_img, wdeg_img


# ----------------------------------------------------------------------------
# Bass program
# ----------------------------------------------------------------------------

def build_program(st, L, D=128):
    W = st["W"]
    NP = st["NP"]
    NT = st["NT"]
    HALF = st["HALF"]
    C = st["C"]
    nblk = st["nblk"]
    KDEG = st["KDEG"]
    NBLK = int(nblk.sum())
    TOT = NBLK * 128
    IDXW = TOT // 16
    KCOLS = int(KDEG.sum())

    nc = bacc.Bacc("TRN2", target_bir_lowering=False, debug=True,
                   num_swdge_queues=4)

    x_in = nc.dram_tensor("x_shard", [NP, D], F32, kind="ExternalInput")
    idx_in = nc.dram_tensor("idx_img", [128, IDXW], I16, kind="ExternalInput")
    s_in = nc.dram_tensor("s_img", [128, TOT], BF16, kind="ExternalInput")
    wdeg_in = nc.dram_tensor("wdeg_img", [128, KCOLS], F32, kind="ExternalInput")
    wst_in = nc.dram_tensor("wst", [L, D, D], F32, kind="ExternalInput")
    bias_in = nc.dram_tensor("bias_b", [L, D, D], F32, kind="ExternalInput")
    gam_in = nc.dram_tensor("gamma_b", [L, D, D], F32, kind="ExternalInput")
    bet_in = nc.dram_tensor("beta_b", [L, D, D], F32, kind="ExternalInput")
    id_in = nc.dram_tensor("ident", [D, D], F32, kind="ExternalInput")
    out_t = nc.dram_tensor("out_shard", [NP, D], F32, kind="ExternalOutput")

    with TileContext(nc) as tc:
        with (
            tc.tile_pool(name="persist", bufs=1) as pp,
            tc.tile_pool(name="gath", bufs=3) as gp,
            tc.tile_pool(name="smat", bufs=3) as sp,
            tc.tile_pool(name="work", bufs=3) as wk,
            tc.tile_pool(name="tiny", bufs=4) as tn,
            tc.tile_pool(name="psum", bufs=2, space="PSUM") as ps,
            tc.tile_pool(name="psagg", bufs=4, space="PSUM") as pagg,
            tc.tile_pool(name="dram", bufs=1, space="DRAM") as dr,
        ):
            # ---- persistent SBUF state ----
            h = pp.tile([128, W, D], F32, tag="h")
            idx = pp.tile([128, IDXW], I16, tag="idx")
            wdeg = pp.tile([128, KCOLS], F32, tag="wdeg")
            wst = pp.tile([128, L * D], F32, tag="wst")
            biasb = pp.tile([128, L * D], F32, tag="biasb")
            gamb = pp.tile([128, L * D], F32, tag="gamb")
            betb = pp.tile([128, L * D], F32, tag="betb")
            ident = pp.tile([128, D], F32, tag="ident")
            dinv = pp.tile([128, W], F32, tag="dinv")

            nc.sync.dma_start(out=h[:, :, :],
                              in_=x_in[:].rearrange("(w p) f -> p w f", p=128))
            nc.sync.dma_start(out=idx[:, :], in_=idx_in[:, :])
            nc.sync.dma_start(out=wdeg[:, :], in_=wdeg_in[:, :])
            for l in range(L):
                for dst_t, src_t in ((wst, wst_in), (biasb, bias_in),
                                     (gamb, gam_in), (betb, bet_in)):
                    nc.sync.dma_start(out=dst_t[:, l * D:(l + 1) * D],
                                      in_=src_t[l, :, :])
            nc.sync.dma_start(out=ident[:, :], in_=id_in[:, :])

            nc.gpsimd.load_library(mlp_library)

            # ---- degree -> dinv (once; includes self-loop weights) ----
            deg = tn.tile([128, W], F32, tag="deg")
            off = 0
            for w in range(W):
                K = int(KDEG[w])
                nc.vector.tensor_reduce(deg[:, w:w + 1], wdeg[:, off:off + K],
                                        AX.X, ALU.add)
                off += K
            rdeg = tn.tile([128, W], F32, tag="rdeg")
            nc.vector.reciprocal(rdeg[:, :], deg[:, :])
            nc.scalar.sqrt(dinv[:, :], rdeg[:, :])

            # ---- per-layer DRAM tables (double buffered across layers) ----
            tables = [dr.tile([NT, D], BF16, name=f"table{i}", tag=f"table{i}",
                              addr_space="Local") for i in range(L)]
            xw_own = [dr.tile([NP, D], BF16, name=f"xwown{i}", tag=f"xwown{i}")
                      for i in range(2)]

            for li in range(L):
                tab = tables[li]
                own = xw_own[li % 2]
                wst_l = wst[:, li * D:(li + 1) * D]
                # -- build own table shard: T = bf16(dinv * (h @ Ws^T)) --
                for w in range(W):
                    hT = ps.tile([128, D], F32, tag="hT")
                    nc.tensor.transpose(hT[:, :], h[:, w, :], ident[:, :])
                    hTs = wk.tile([128, D], F32, tag="hTs")
                    nc.scalar.activation(hTs[:, :], hT[:, :], ACTF.Copy)
                    mm = ps.tile([128, D], F32, tag="mm")
                    nc.tensor.matmul(mm[:, :], hTs[:, :], wst_l)
                    xwb = wk.tile([128, D], BF16, tag="xwb")
                    nc.scalar.activation(xwb[:, :], mm[:, :], ACTF.Copy,
                                         scale=dinv[:, w:w + 1])
                    nc.sync.dma_start(out=own[w * 128:(w + 1) * 128, :],
                                      in_=xwb[:, :])
                nc.gpsimd.collective_compute(
                    "AllGather", ALU.bypass,
                    replica_groups=[list(range(C))],
                    ins=[own[:].opt()], outs=[tab[:].opt()])

                # -- aggregate into owned dests --
                pos = 0
                qn = 0
                for w in range(W):
                    nA, nB = int(nblk[w, 0]), int(nblk[w, 1])
                    nT = nA + nB
                    g = gp.tile([128, nT, D], BF16, tag="g")
                    if nA:
                        nc.gpsimd.dma_gather(
                            g[:, 0:nA, :], tab[:, :],
                            idx[:, pos * 8:(pos + nA) * 8],
                            nA * 128, nA * 128, D, single_packet=False,
                            queue_num=qn % 4)
                        qn += 1
                    if nB:
                        nc.gpsimd.dma_gather(
                            g[:, nA:nT, :], tab[HALF:, :],
                            idx[:, (pos + nA) * 8:(pos + nT) * 8],
                            nB * 128, nB * 128, D, single_packet=False,
                            queue_num=qn % 4)
                        qn += 1
                    s_t = sp.tile([128, nT, 128], BF16, tag="s_t")
                    nc.sync.dma_start(
                        out=s_t[:, :, :],
                        in_=s_in[:, pos * 128:(pos + nT) * 128])
                    agg = pagg.tile([128, D], F32, tag="agg")
                    for b in range(nT):
                        nc.tensor.matmul(agg[:, :], s_t[:, b, :], g[:, b, :],
                                         start=(b == 0), stop=(b == nT - 1))
                    # x0 = agg*dinv + bias
                    x0 = wk.tile([128, D], F32, tag="x0")
                    nc.scalar.activation(x0[:, :], agg[:, :], ACTF.Copy,
                                         scale=dinv[:, w:w + 1])
                    nc.vector.tensor_add(x0[:, :], x0[:, :],
                                         biasb[:, li * D:(li + 1) * D])
                    # layernorm
                    sx = tn.tile([128, 1], F32, tag="sx")
                    nc.vector.tensor_reduce(sx[:, :], x0[:, :], AX.X, ALU.add)
                    sq = tn.tile([128, 1], F32, tag="sq")
                    sqs = wk.tile([128, D], F32, tag="sqs")
                    nc.scalar.activation(sqs[:, :], x0[:, :], ACTF.Square,
                                         accum_out=sq[:, :])
                    negmu = tn.tile([128, 1], F32, tag="negmu")
                    nc.vector.tensor_scalar_mul(negmu[:, :], sx[:, :], -1.0 / D)
                    ms = tn.tile([128, 1], F32, tag="ms")
                    nc.vector.tensor_scalar(ms[:, :], sq[:, :], 1.0 / D,
                                            1e-5, ALU.mult, ALU.add)
                    mu2 = tn.tile([128, 1], F32, tag="mu2")
                    nc.vector.tensor_mul(mu2[:, :], negmu[:, :], negmu[:, :])
                    var = tn.tile([128, 1], F32, tag="var")
                    nc.vector.tensor_sub(var[:, :], ms[:, :], mu2[:, :])
                    rv = tn.tile([128, 1], F32, tag="rv")
                    nc.vector.reciprocal(rv[:, :], var[:, :])
                    rstd = tn.tile([128, 1], F32, tag="rstd")
                    nc.scalar.sqrt(rstd[:, :], rv[:, :])
                    nnmr = tn.tile([128, 1], F32, tag="nnmr")
                    nc.vector.tensor_mul(nnmr[:, :], negmu[:, :], rstd[:, :])
                    t = wk.tile([128, D], F32, tag="t")
                    nc.scalar.activation(t[:, :], x0[:, :], ACTF.Identity,
                                         scale=rstd[:, :], bias=nnmr[:, :])
                    nc.vector.tensor_mul(t[:, :], t[:, :],
                                         gamb[:, li * D:(li + 1) * D])
                    nc.vector.tensor_add(t[:, :], t[:, :],
                                         betb[:, li * D:(li + 1) * D])
                    if li < L - 1:
                        nc.scalar.activation(t[:, :], t[:, :], ACTF.Relu)
                    nc.vector.tensor_add(h[:, w, :], t[:, :], h[:, w, :])
                    pos += nT

            nc.sync.dma_start(out=out_t[:].rearrange("(w p) f -> p w f", p=128),
                              in_=h[:, :, :])

    nc.compile()
    return nc


# ----------------------------------------------------------------------------
# Full kernel entry
# ----------------------------------------------------------------------------

def _kernel_impl(x, edge_index, edge_weight, Ws, bs, gammas, betas,
                 C=8, W=49, HALF=32768, trace=False):
    N, D = x.shape
    L = Ws.shape[0]
    st = build_structure(edge_index, edge_weight, N, C, W, HALF)
    NP, NPC = st["NP"], st["NPC"]

    ident = np.eye(D, dtype=np.float32)
    wst = np.ascontiguousarray(np.transpose(np.asarray(Ws), (0, 2, 1))).astype(np.float32)
    bias_b = np.ascontiguousarray(
        np.broadcast_to(np.asarray(bs)[:, None, :], (L, D, D))).astype(np.float32)
    gam_b = np.ascontiguousarray(
        np.broadcast_to(np.asarray(gammas)[:, None, :], (L, D, D))).astype(np.float32)
    bet_b = np.ascontiguousarray(
        np.broadcast_to(np.asarray(betas)[:, None, :], (L, D, D))).astype(np.float32)

    in_maps = []
    for c in range(C):
        idx_img, s_img, wdeg_img = pack_core(st, c)
        xs = np.zeros((NP, D), dtype=np.float32)
        lo = c * NPC
        xs[st["rank"][lo:lo + NPC]] = np.asarray(x[lo:lo + NPC], dtype=np.float32)
        in_maps.append(dict(x_shard=xs, idx_img=idx_img, s_img=s_img,
                            wdeg_img=wdeg_img, wst=wst, bias_b=bias_b,
                            gamma_b=gam_b, beta_b=bet_b, ident=ident))

    nc = build_program(st, L, D)
    res = run_bass_kernel_spmd(nc, in_maps, list(range(C)), trace=trace)

    out = np.empty((N, D), dtype=np.float32)
    for c in range(C):
        lo = c * NPC
        sh = res.results[c]["out_shard"]
        out[lo:lo + NPC] = sh[st["rank"][lo:lo + NPC]]
    return out, res


def kernel(x, edge_index, edge_weight, Ws, bs, gammas, betas):
    out, _ = _kernel_impl(np.asarray(x), np.asarray(edge_index),
                          np.asarray(edge_weight), np.asarray(Ws),
                          np.asarray(bs), np.asarray(gammas), np.asarray(betas))
    return out
